# revision 9
# baseline (speedup 1.0000x reference)
"""Trainium2 Bass kernel for nn_E3nnMLPNorm (8-core SPMD).

Structure exploited: the input irreps are '2x1e' and every linear is
block-diagonal per irrep, so the l=2,3,4 fields are exactly zero through
the whole network (bn_act(0) == 0). Only the l=1 (d=3) path is computed.

Sharding: data-parallel over batch B=1024 -> 128 rows/core. Activations
live as [v(128-partition tiles), col = i*128 + b_local].

Per layer (1..5): mm = f @ W on PE as fp16 hi/lo 3-pass (fp32 PSUM
accumulate), ordered in u-phases {0,1},{2,3},{4..7} so the next layer can
start as soon as the first normalized u-tiles arrive. Banks complete
t-major inside the last phase; each finished bank is immediately drained
to SBUF (freeing PSUM for the next layer) while ACT computes sum(mm^2).
Batch-norm statistics cross cores via 3 pipelined AllGathers (4/2/2KB) +
local reduce — cheaper floor than AllReduce and overlapped with the PE.
Layer 0 needs no collective at all: var0 is a quadratic form in the
2x2 Gram matrix of the full x, computed locally on every core.
"""
import sys, types
sys.path.insert(0, "/opt/trn_rl_repo")
import numpy as np

# ---- shim antenv.axon_hooks so trace=True works under axon ----
if "antenv.axon_hooks" not in sys.modules:
    _hook_store = {}
    _m = types.ModuleType("antenv.axon_hooks")
    _m.set_axon_ntff_profile_hook = lambda h: _hook_store.__setitem__("h", h)
    _m.get_axon_ntff_profile_hook = lambda: _hook_store.get("h")
    sys.modules["antenv.axon_hooks"] = _m
    try:
        import antenv
        antenv.axon_hooks = _m
        from trn_agent_boot.trn_boot import _ntff_profile_via_ctypes
        _m.set_axon_ntff_profile_hook(
            _ntff_profile_via_ctypes("/opt/axon/libaxon_pjrt.so"))
    except Exception:
        pass

import concourse.bass as bass
import concourse.bacc as bacc
import concourse.mybir as mybir
import concourse.tile as tile
import concourse.bass_utils as bass_utils
bass_utils.upload_artifacts = lambda tmpdir: tmpdir
from concourse.bass_utils import run_bass_kernel_spmd

F32 = mybir.dt.float32
F16 = mybir.dt.float16
ALU = mybir.AluOpType
ACT = mybir.ActivationFunctionType

NCORE = 8
MUL = 1024
NT = 8
BSH = 128
COLS = 3 * BSH
NLAYER = 6

# stat groups (tile ranges) and matching u-phases
GROUPS = [(0, 2), (2, 4), (4, 8)]
PHASES = [(0, 2), (2, 4), (4, 8)]

EPS_L = [2e-5] + [1.024e-2] * 5      # rsqrt eps with 1/sqrt(mul) folded in
INV_SAMPLES = 1.0 / (MUL * 3)


def _build():
    nc = bacc.Bacc("TRN2", target_bir_lowering=False, debug=False,
                   enable_asserts=True, num_devices=NCORE)

    d_x = nc.dram_tensor("x_c", [2, COLS], F32, kind="ExternalInput")
    d_xq = nc.dram_tensor("xq", [24, 128, 2], F32, kind="ExternalInput")
    d_w1 = nc.dram_tensor("w1", [2, MUL], F32, kind="ExternalInput")
    d_w1T = nc.dram_tensor("w1T", [NT, 128, 2], F32, kind="ExternalInput")
    d_wh = nc.dram_tensor("wh", [5, NT, 128, MUL], F16, kind="ExternalInput")
    d_wl = nc.dram_tensor("wl", [5, NT, 128, MUL], F16, kind="ExternalInput")
    d_bnw = nc.dram_tensor("bnw", [NLAYER, 128, NT], F32, kind="ExternalInput")
    d_wout = nc.dram_tensor("woutT", [128, NT], F32, kind="ExternalInput")
    d_out = nc.dram_tensor("out", [1, COLS], F32, kind="ExternalOutput")

    with tile.TileContext(nc) as tc:
        with tc.tile_pool(name="const", bufs=1) as constp, \
             tc.tile_pool(name="wpool", bufs=2) as wpool, \
             tc.tile_pool(name="acts", bufs=2) as acts, \
             tc.tile_pool(name="sqp", bufs=3) as sqp, \
             tc.tile_pool(name="stats", bufs=3) as stats, \
             tc.tile_pool(name="gate", bufs=2) as gatep, \
             tc.tile_pool(name="psum", bufs=1, space="PSUM") as psump, \
             tc.tile_pool(name="dram", bufs=1, space="DRAM") as dramp:

            # ---- static loads ----
            x_sb = constp.tile([2, COLS], F32, tag="x")
            nc.sync.dma_start(x_sb[:], d_x[:])
            xq_sb = constp.tile([128, 24, 2], F32, tag="xq")
            nc.sync.dma_start(xq_sb[:], d_xq.rearrange("c p j -> p c j"))
            w1_sb = constp.tile([2, MUL], F32, tag="w1")
            nc.sync.dma_start(w1_sb[:], d_w1[:])
            w1T_sb = constp.tile([128, NT, 2], F32, tag="w1T")
            nc.sync.dma_start(w1T_sb[:], d_w1T.rearrange("t p j -> p t j"))
            bnw_sb = constp.tile([128, NLAYER, NT], F32, tag="bnw")
            nc.sync.dma_start(bnw_sb[:], d_bnw.rearrange("l p t -> p l t"))
            wout_sb = constp.tile([128, NT], F32, tag="wout")
            nc.sync.dma_start(wout_sb[:], d_wout[:])
            eps8 = constp.tile([128, 1], F32, tag="eps8")
            nc.vector.memset(eps8[:], 1e-8)

            ps = psump.tile([128, NT, 512], F32, tag="ps")

            def load_w(b):
                wh = wpool.tile([128, NT, MUL], F16, tag="wh")
                nc.sync.dma_start(wh[:], d_wh[b].rearrange("t p v -> p t v"))
                wl = wpool.tile([128, NT, MUL], F16, tag="wl")
                nc.sync.dma_start(wl[:], d_wl[b].rearrange("t p v -> p t v"))
                return wh, wl

            # ---------- layer 0: Gram-trick stats (no collective) ----------
            # S = sum_c x x^T over all 3072 samples, via 24 K=128 matmuls
            for c in range(24):
                nc.tensor.matmul(ps[0:2, 0, 0:2], xq_sb[:, c, :],
                                 xq_sb[:, c, :], start=(c == 0),
                                 stop=(c == 23))
            S_sb = stats.tile([2, 2], F32, tag="S")
            nc.vector.tensor_copy(S_sb[:], ps[0:2, 0, 0:2])
            sv = stats.tile([1, 3], F32, tag="sv")
            nc.sync.dma_start(sv[0:1, 0:2], S_sb[0:1, 0:2])
            nc.sync.dma_start(sv[0:1, 2:3], S_sb[1:2, 1:2])
            S_brd = stats.tile([128, 3], F32, tag="Sbrd")
            nc.gpsimd.partition_broadcast(S_brd[:], sv[0:1, :])
            A = w1T_sb[:, :, 0]
            B = w1T_sb[:, :, 1]
            AA = stats.tile([128, NT], F32, tag="AA")
            nc.vector.tensor_mul(AA[:], A, A)
            AB2 = stats.tile([128, NT], F32, tag="AB2")
            nc.vector.scalar_tensor_tensor(out=AB2[:], in0=A, scalar=2.0,
                                           in1=B, op0=ALU.mult, op1=ALU.mult)
            BB = stats.tile([128, NT], F32, tag="BB")
            nc.vector.tensor_mul(BB[:], B, B)
            z0 = stats.tile([128, NT], F32, tag="z0")
            nc.vector.tensor_scalar_mul(out=z0[:], in0=AA[:],
                                        scalar1=S_brd[:, 0:1])
            nc.vector.scalar_tensor_tensor(out=z0[:], in0=AB2[:],
                                           scalar=S_brd[:, 1:2], in1=z0[:],
                                           op0=ALU.mult, op1=ALU.add)
            nc.vector.scalar_tensor_tensor(out=z0[:], in0=BB[:],
                                           scalar=S_brd[:, 2:3], in1=z0[:],
                                           op0=ALU.mult, op1=ALU.add)

            # layer-0 matmul: [2]x[2,128v] fp32
            for t in range(NT):
                nc.tensor.matmul(ps[:, t, 0:COLS],
                                 w1_sb[:, t * 128:(t + 1) * 128],
                                 x_sb[:], start=True, stop=True)

            # ---------- shared per-layer pieces ----------
            def drain_stats(t, mm_sb, m2, vp, vcol):
                mm_t = ps[:, t, 0:COLS]
                sq = sqp.tile([128, COLS], F32, tag="sq")
                if vp is not None:
                    nc.scalar.activation(sq[:], mm_t, ACT.Square,
                                         accum_out=vp[:, vcol:vcol + 1])
                else:
                    nc.scalar.activation(sq[:], mm_t, ACT.Square)
                nc.scalar.copy(mm_sb[:, t, :], mm_t)
                nc.vector.tensor_reduce(
                    out=m2[:, t * 128:(t + 1) * 128],
                    in_=sq.rearrange("p (i b) -> p b i", i=3),
                    axis=mybir.AxisListType.X, op=ALU.add)

            def gate_group(l, z_g, t0, t1, m2, mm_sb, g, fh, fl,
                           produce_split):
                ng = t1 - t0
                zz = stats.tile([128, ng], F32, tag="zz")
                nc.vector.tensor_scalar(out=zz[:], in0=z_g,
                                        scalar1=INV_SAMPLES, scalar2=EPS_L[l],
                                        op0=ALU.mult, op1=ALU.add)
                rr = stats.tile([128, ng], F32, tag="rr")
                nc.vector.reciprocal(rr[:], zz[:])
                spre = stats.tile([128, ng], F32, tag="spre")
                nc.scalar.sqrt(spre[:], rr[:])
                s_g = stats.tile([128, ng], F32, tag="sg")
                nc.vector.tensor_mul(s_g[:], spre[:], bnw_sb[:, l, t0:t1])
                s2_g = stats.tile([128, ng], F32, tag="s2g")
                nc.vector.tensor_mul(s2_g[:], s_g[:], s_g[:])

                gc = slice(t0 * 128, t1 * 128)
                t1v = gatep.tile([128, ng * 128], F32, tag="t1v")
                for i, t in enumerate(range(t0, t1)):
                    nc.vector.tensor_scalar_mul(
                        out=t1v[:, i * 128:(i + 1) * 128],
                        in0=m2[:, t * 128:(t + 1) * 128],
                        scalar1=s2_g[:, i:i + 1])
                n_g = gatep.tile([128, ng * 128], F32, tag="ng")
                nc.scalar.activation(n_g[:], t1v[:], ACT.Sqrt,
                                     bias=eps8[:], scale=1.0)
                sig = gatep.tile([128, ng * 128], F32, tag="sig")
                nc.scalar.activation(sig[:], n_g[:], ACT.Sigmoid)
                rn = gatep.tile([128, ng * 128], F32, tag="rn")
                nc.vector.reciprocal(rn[:], n_g[:])
                q1 = gatep.tile([128, ng * 128], F32, tag="q1")
                nc.vector.tensor_mul(q1[:], sig[:], rn[:])
                for i, t in enumerate(range(t0, t1)):
                    Qc = gatep.tile([128, 128], F32, tag="Qc")
                    nc.vector.tensor_scalar_mul(
                        out=Qc[:], in0=q1[:, i * 128:(i + 1) * 128],
                        scalar1=s_g[:, i:i + 1])
                    mm_v = mm_sb[:, t, :].rearrange("p (i b) -> p i b", i=3)
                    g_v = g[:, t, :].rearrange("p (i b) -> p i b", i=3)
                    qb = Qc[:, None, :].broadcast_to([128, 3, 128])
                    nc.vector.tensor_tensor(out=g_v, in0=mm_v, in1=qb,
                                            op=ALU.mult)
                    if produce_split:
                        nc.scalar.copy(out=fh[:, t, :], in_=g[:, t, :])
                        nc.vector.tensor_sub(out=fl[:, t, :], in0=g[:, t, :],
                                             in1=fh[:, t, :])

            # ---------- layer 0 normalize (local stats, 4 groups) ----------
            mm_sb = acts.tile([128, NT, COLS], F32, tag="mm")
            m2 = gatep.tile([128, MUL], F32, tag="m2")
            g = acts.tile([128, NT, COLS], F32, tag="g")
            fh = acts.tile([128, NT, COLS], F16, tag="fh")
            fl = acts.tile([128, NT, COLS], F16, tag="fl")
            for (t0, t1) in [(0, 2), (2, 4), (4, 6), (6, 8)]:
                for t in range(t0, t1):
                    drain_stats(t, mm_sb, m2, None, 0)
                gate_group(0, z0[:, t0:t1], t0, t1, m2, mm_sb, g, fh, fl,
                           True)

            # ---------- layers 1..5 ----------
            for b in range(5):
                l = b + 1
                wh, wl = load_w(b)
                mm_sb_n = acts.tile([128, NT, COLS], F32, tag="mm")
                m2_n = gatep.tile([128, MUL], F32, tag="m2")
                vps = [stats.tile([128, t1 - t0], F32, tag=f"vp{gi}",
                                  name=f"vp{l}_{gi}")
                       for gi, (t0, t1) in enumerate(GROUPS)]
                ag_outs = []
                for (u0, u1) in PHASES:
                    last_phase = (u1 == NT)
                    for t in range(NT):
                        for u in range(u0, u1):
                            lh = wh[:, u, t * 128:(t + 1) * 128]
                            ll = wl[:, u, t * 128:(t + 1) * 128]
                            nc.tensor.matmul(ps[:, t, 0:COLS], lh,
                                             fh[:, u, :],
                                             start=(u == 0), stop=False)
                            nc.tensor.matmul(ps[:, t, 0:COLS], lh,
                                             fl[:, u, :],
                                             start=False, stop=False)
                            nc.tensor.matmul(ps[:, t, 0:COLS], ll,
                                             fh[:, u, :],
                                             start=False, stop=(u == NT - 1))
                        if last_phase:
                            gi = next(i for i, (t0, t1) in enumerate(GROUPS)
                                      if t0 <= t < t1)
                            t0, t1 = GROUPS[gi]
                            drain_stats(t, mm_sb_n, m2_n, vps[gi], t - t0)
                            if t == t1 - 1:
                                ngr = t1 - t0
                                ag_i = dramp.tile([128, ngr], F32,
                                                  tag=f"agi{l}_{gi}")
                                ag_o = dramp.tile([NCORE, 128, ngr], F32,
                                                  tag=f"ago{l}_{gi}")
                                nc.scalar.dma_start(ag_i[:], vps[gi][:])
                                nc.gpsimd.collective_compute(
                                    "AllGather", ALU.bypass,
                                    replica_groups=[list(range(NCORE))],
                                    ins=[ag_i.opt()], outs=[ag_o.opt()])
                                ag_outs.append(ag_o)

                g_n = acts.tile([128, NT, COLS], F32, tag="g")
                fh_n = acts.tile([128, NT, COLS], F16, tag="fh")
                fl_n = acts.tile([128, NT, COLS], F16, tag="fl")
                for gi, (t0, t1) in enumerate(GROUPS):
                    ngr = t1 - t0
                    zcat = stats.tile([128, NCORE, ngr], F32, tag="zcat")
                    nc.gpsimd.dma_start(zcat[:],
                                        ag_outs[gi].rearrange("r p g -> p r g"))
                    z_g = stats.tile([128, ngr], F32, tag="zg")
                    nc.vector.tensor_reduce(
                        out=z_g[:], in_=zcat.rearrange("p r g -> p g r"),
                        axis=mybir.AxisListType.X, op=ALU.add)
                    gate_group(l, z_g[:], t0, t1, m2_n, mm_sb_n,
                               g_n, fh_n, fl_n, produce_split=(l < 5))
                mm_sb, m2, g, fh, fl = mm_sb_n, m2_n, g_n, fh_n, fl_n

            # ---------- output layer ----------
            for u in range(NT):
                nc.tensor.matmul(ps[0:1, 0, 0:COLS], wout_sb[:, u:u + 1],
                                 g[:, u, :], start=(u == 0), stop=(u == NT - 1))
            o_sb = stats.tile([1, COLS], F32, tag="o")
            nc.vector.tensor_copy(o_sb[:], ps[0:1, 0, 0:COLS])
            nc.sync.dma_start(d_out[:], o_sb[:])

    nc.compile()
    return nc


_NC = None


def _get_nc():
    global _NC
    if _NC is None:
        _NC = _build()
    return _NC


def _prep_in_maps(x, w1, W, bn_w, w_out):
    B = 1024
    xbui = x.reshape(B, 2, 3).astype(np.float32)
    xx = np.ascontiguousarray(xbui.transpose(1, 2, 0))     # [u, i, b]
    xq = np.ascontiguousarray(
        xbui.transpose(0, 2, 1).reshape(3072, 2).reshape(24, 128, 2))
    w1f = np.ascontiguousarray(w1.astype(np.float32))
    w1T = np.ascontiguousarray(w1f.T.reshape(NT, 128, 2))
    W0 = np.ascontiguousarray(W[:, 0].astype(np.float32))  # [5, u, v]
    Wh = W0.astype(np.float16)
    Wl = (W0 - Wh.astype(np.float32)).astype(np.float16)
    Wh = np.ascontiguousarray(Wh.reshape(5, NT, 128, MUL))
    Wl = np.ascontiguousarray(Wl.reshape(5, NT, 128, MUL))
    bnw = np.ascontiguousarray(
        bn_w[:, 0].astype(np.float32).reshape(NLAYER, NT, 128)
        .transpose(0, 2, 1))
    woutT = np.ascontiguousarray(
        (w_out[:, 0].astype(np.float32) / 32.0).reshape(NT, 128).T)

    in_maps = []
    for c in range(NCORE):
        x_c = np.ascontiguousarray(
            xx[:, :, c * BSH:(c + 1) * BSH].reshape(2, COLS))
        in_maps.append({"x_c": x_c, "xq": xq, "w1": w1f, "w1T": w1T,
                        "wh": Wh, "wl": Wl, "bnw": bnw, "woutT": woutT})
    return in_maps


def _run(inputs, trace=False, trace_cores=None):
    nc = _get_nc()
    in_maps = _prep_in_maps(inputs["x"], inputs["w1"], inputs["W"],
                            inputs["bn_w"], inputs["w_out"])
    res = run_bass_kernel_spmd(nc, in_maps, core_ids=list(range(NCORE)),
                               trace=trace, trace_cores=trace_cores)
    out = np.empty((1024, 3), dtype=np.float32)
    for c in range(NCORE):
        o = res.results[c]["out"].reshape(3, 128).T
        out[c * BSH:(c + 1) * BSH] = o
    return out, res


def kernel(**inputs) -> np.ndarray:
    out, _ = _run(inputs, trace=False)
    return out


# revision 12
# speedup vs baseline: 1.2073x; 1.2073x over previous
"""Trainium2 Bass kernel for nn_E3nnMLPNorm (8-core SPMD).

Structure exploited: the input irreps are '2x1e' and every linear is
block-diagonal per irrep, so the l=2,3,4 fields are exactly zero through
the whole network (bn_act(0) == 0). Only the l=1 (d=3) path is computed.

Sharding: data-parallel over batch B=1024 -> 128 rows/core. Activations
live as [v(128-partition tiles), col = i*128 + b_local].

Per layer (1..5): mm = f @ W on PE as fp16 hi/lo 3-pass (fp32 PSUM
accumulate), ordered in u-phases {0,1},{2,3},{4..7} so the next layer can
start as soon as the first normalized u-tiles arrive. Banks complete
t-major inside the last phase; each finished bank is immediately drained
to SBUF (freeing PSUM for the next layer) while ACT computes sum(mm^2).
Batch-norm statistics cross cores via 3 pipelined AllGathers (4/2/2KB) +
local reduce — cheaper floor than AllReduce and overlapped with the PE.
Layer 0 needs no collective at all: var0 is a quadratic form in the
2x2 Gram matrix of the full x, computed locally on every core.
"""
import sys, types
sys.path.insert(0, "/opt/trn_rl_repo")
import numpy as np

# ---- shim antenv.axon_hooks so trace=True works under axon ----
if "antenv.axon_hooks" not in sys.modules:
    _hook_store = {}
    _m = types.ModuleType("antenv.axon_hooks")
    _m.set_axon_ntff_profile_hook = lambda h: _hook_store.__setitem__("h", h)
    _m.get_axon_ntff_profile_hook = lambda: _hook_store.get("h")
    sys.modules["antenv.axon_hooks"] = _m
    try:
        import antenv
        antenv.axon_hooks = _m
        from trn_agent_boot.trn_boot import _ntff_profile_via_ctypes
        _m.set_axon_ntff_profile_hook(
            _ntff_profile_via_ctypes("/opt/axon/libaxon_pjrt.so"))
    except Exception:
        pass

import concourse.bass as bass
import concourse.bacc as bacc
import concourse.mybir as mybir
import concourse.tile as tile
import concourse.bass_utils as bass_utils
bass_utils.upload_artifacts = lambda tmpdir: tmpdir
from concourse.bass_utils import run_bass_kernel_spmd

F32 = mybir.dt.float32
F16 = mybir.dt.float16
ALU = mybir.AluOpType
ACT = mybir.ActivationFunctionType

NCORE = 8
MUL = 1024
NT = 8
BSH = 128
COLS = 3 * BSH
NLAYER = 6

# stat groups (tile ranges) and matching u-phases
GROUPS = [(0, 2), (2, 4), (4, 8)]
PHASES = [(0, 2), (2, 4), (4, 8)]

EPS_L = [2e-5] + [1.024e-2] * 5      # rsqrt eps with 1/sqrt(mul) folded in
INV_SAMPLES = 1.0 / (MUL * 3)


def _build():
    nc = bacc.Bacc("TRN2", target_bir_lowering=False, debug=False,
                   enable_asserts=True, num_devices=NCORE)

    d_x = nc.dram_tensor("x_c", [2, COLS], F32, kind="ExternalInput")
    d_xq = nc.dram_tensor("xq", [24, 128, 2], F32, kind="ExternalInput")
    d_w1 = nc.dram_tensor("w1", [2, MUL], F32, kind="ExternalInput")
    d_w1T = nc.dram_tensor("w1T", [NT, 128, 2], F32, kind="ExternalInput")
    d_wh = nc.dram_tensor("wh", [5, NT, 128, MUL], F16, kind="ExternalInput")
    d_wl = nc.dram_tensor("wl", [5, NT, 128, MUL], F16, kind="ExternalInput")
    d_bnw = nc.dram_tensor("bnw", [NLAYER, 128, NT], F32, kind="ExternalInput")
    d_wout = nc.dram_tensor("woutT", [128, NT], F32, kind="ExternalInput")
    d_out = nc.dram_tensor("out", [1, COLS], F32, kind="ExternalOutput")

    with tile.TileContext(nc) as tc:
        with tc.tile_pool(name="const", bufs=1) as constp, \
             tc.tile_pool(name="wpool", bufs=2) as wpool, \
             tc.tile_pool(name="acts", bufs=2) as acts, \
             tc.tile_pool(name="sqp", bufs=3) as sqp, \
             tc.tile_pool(name="stats", bufs=3) as stats, \
             tc.tile_pool(name="gate", bufs=2) as gatep, \
             tc.tile_pool(name="psum", bufs=1, space="PSUM") as psump, \
             tc.tile_pool(name="dram", bufs=1, space="DRAM") as dramp:

            # align cores up-front so the first real collective doesn't
            # absorb the cross-core launch skew
            sync_in = dramp.tile([128, 1], F32, tag="sync_i")
            sync_out = dramp.tile([NCORE, 128, 1], F32, tag="sync_o")
            nc.gpsimd.collective_compute(
                "AllGather", ALU.bypass,
                replica_groups=[list(range(NCORE))],
                ins=[sync_in.opt()], outs=[sync_out.opt()])

            # ---- static loads ----
            x_sb = constp.tile([2, COLS], F32, tag="x")
            nc.sync.dma_start(x_sb[:], d_x[:])
            xq_sb = constp.tile([128, 24, 2], F32, tag="xq")
            nc.sync.dma_start(xq_sb[:], d_xq.rearrange("c p j -> p c j"))
            w1_sb = constp.tile([2, MUL], F32, tag="w1")
            nc.sync.dma_start(w1_sb[:], d_w1[:])
            w1T_sb = constp.tile([128, NT, 2], F32, tag="w1T")
            nc.sync.dma_start(w1T_sb[:], d_w1T.rearrange("t p j -> p t j"))
            bnw_sb = constp.tile([128, NLAYER, NT], F32, tag="bnw")
            nc.sync.dma_start(bnw_sb[:], d_bnw.rearrange("l p t -> p l t"))
            wout_sb = constp.tile([128, NT], F32, tag="wout")
            nc.sync.dma_start(wout_sb[:], d_wout[:])
            eps8 = constp.tile([128, 1], F32, tag="eps8")
            nc.vector.memset(eps8[:], 1e-8)

            ps = psump.tile([128, NT, 512], F32, tag="ps")

            def load_w(b):
                wh = wpool.tile([128, NT, MUL], F16, tag="wh")
                nc.sync.dma_start(wh[:], d_wh[b].rearrange("t p v -> p t v"))
                wl = wpool.tile([128, NT, MUL], F16, tag="wl")
                nc.sync.dma_start(wl[:], d_wl[b].rearrange("t p v -> p t v"))
                return wh, wl

            # ---------- layer 0: Gram-trick stats (no collective) ----------
            # S = sum_c x x^T over all 3072 samples, via 24 K=128 matmuls
            for c in range(24):
                nc.tensor.matmul(ps[0:2, 0, 0:2], xq_sb[:, c, :],
                                 xq_sb[:, c, :], start=(c == 0),
                                 stop=(c == 23))
            S_sb = stats.tile([2, 2], F32, tag="S")
            nc.vector.tensor_copy(S_sb[:], ps[0:2, 0, 0:2])
            sv = stats.tile([1, 3], F32, tag="sv")
            nc.sync.dma_start(sv[0:1, 0:2], S_sb[0:1, 0:2])
            nc.sync.dma_start(sv[0:1, 2:3], S_sb[1:2, 1:2])
            S_brd = stats.tile([128, 3], F32, tag="Sbrd")
            nc.gpsimd.partition_broadcast(S_brd[:], sv[0:1, :])
            A = w1T_sb[:, :, 0]
            B = w1T_sb[:, :, 1]
            AA = stats.tile([128, NT], F32, tag="AA")
            nc.vector.tensor_mul(AA[:], A, A)
            AB2 = stats.tile([128, NT], F32, tag="AB2")
            nc.vector.scalar_tensor_tensor(out=AB2[:], in0=A, scalar=2.0,
                                           in1=B, op0=ALU.mult, op1=ALU.mult)
            BB = stats.tile([128, NT], F32, tag="BB")
            nc.vector.tensor_mul(BB[:], B, B)
            z0 = stats.tile([128, NT], F32, tag="z0")
            nc.vector.tensor_scalar_mul(out=z0[:], in0=AA[:],
                                        scalar1=S_brd[:, 0:1])
            nc.vector.scalar_tensor_tensor(out=z0[:], in0=AB2[:],
                                           scalar=S_brd[:, 1:2], in1=z0[:],
                                           op0=ALU.mult, op1=ALU.add)
            nc.vector.scalar_tensor_tensor(out=z0[:], in0=BB[:],
                                           scalar=S_brd[:, 2:3], in1=z0[:],
                                           op0=ALU.mult, op1=ALU.add)

            # layer-0 matmul: [2]x[2,128v] fp32
            for t in range(NT):
                nc.tensor.matmul(ps[:, t, 0:COLS],
                                 w1_sb[:, t * 128:(t + 1) * 128],
                                 x_sb[:], start=True, stop=True)

            # ---------- shared per-layer pieces ----------
            def drain_stats(t, mm_sb, m2, vp, vcol):
                mm_t = ps[:, t, 0:COLS]
                sq = sqp.tile([128, COLS], F32, tag="sq")
                if vp is not None:
                    nc.scalar.activation(sq[:], mm_t, ACT.Square,
                                         accum_out=vp[:, vcol:vcol + 1])
                else:
                    nc.scalar.activation(sq[:], mm_t, ACT.Square)
                nc.scalar.copy(mm_sb[:, t, :], mm_t)
                nc.vector.tensor_reduce(
                    out=m2[:, t * 128:(t + 1) * 128],
                    in_=sq.rearrange("p (i b) -> p b i", i=3),
                    axis=mybir.AxisListType.X, op=ALU.add)

            def gate_group(l, z_g, t0, t1, m2, mm_sb, g, fh, fl,
                           produce_split):
                ng = t1 - t0
                zz = stats.tile([128, ng], F32, tag="zz")
                nc.vector.tensor_scalar(out=zz[:], in0=z_g,
                                        scalar1=INV_SAMPLES, scalar2=EPS_L[l],
                                        op0=ALU.mult, op1=ALU.add)
                rr = stats.tile([128, ng], F32, tag="rr")
                nc.vector.reciprocal(rr[:], zz[:])
                spre = stats.tile([128, ng], F32, tag="spre")
                nc.scalar.sqrt(spre[:], rr[:])
                s_g = stats.tile([128, ng], F32, tag="sg")
                nc.vector.tensor_mul(s_g[:], spre[:], bnw_sb[:, l, t0:t1])
                s2_g = stats.tile([128, ng], F32, tag="s2g")
                nc.vector.tensor_mul(s2_g[:], s_g[:], s_g[:])

                gc = slice(t0 * 128, t1 * 128)
                t1v = gatep.tile([128, ng * 128], F32, tag="t1v")
                for i, t in enumerate(range(t0, t1)):
                    nc.vector.tensor_scalar_mul(
                        out=t1v[:, i * 128:(i + 1) * 128],
                        in0=m2[:, t * 128:(t + 1) * 128],
                        scalar1=s2_g[:, i:i + 1])
                n_g = gatep.tile([128, ng * 128], F32, tag="ng")
                nc.scalar.activation(n_g[:], t1v[:], ACT.Sqrt,
                                     bias=eps8[:], scale=1.0)
                sig = gatep.tile([128, ng * 128], F32, tag="sig")
                nc.scalar.activation(sig[:], n_g[:], ACT.Sigmoid)
                rn = gatep.tile([128, ng * 128], F32, tag="rn")
                nc.vector.reciprocal(rn[:], n_g[:])
                q1 = gatep.tile([128, ng * 128], F32, tag="q1")
                nc.vector.tensor_mul(q1[:], sig[:], rn[:])
                for i, t in enumerate(range(t0, t1)):
                    Qc = gatep.tile([128, 128], F32, tag="Qc")
                    nc.vector.tensor_scalar_mul(
                        out=Qc[:], in0=q1[:, i * 128:(i + 1) * 128],
                        scalar1=s_g[:, i:i + 1])
                    mm_v = mm_sb[:, t, :].rearrange("p (i b) -> p i b", i=3)
                    g_v = g[:, t, :].rearrange("p (i b) -> p i b", i=3)
                    qb = Qc[:, None, :].broadcast_to([128, 3, 128])
                    nc.vector.tensor_tensor(out=g_v, in0=mm_v, in1=qb,
                                            op=ALU.mult)
                    if produce_split:
                        nc.scalar.copy(out=fh[:, t, :], in_=g[:, t, :])
                        nc.vector.tensor_sub(out=fl[:, t, :], in0=g[:, t, :],
                                             in1=fh[:, t, :])

            # ---------- layer 0 normalize (local stats, 4 groups) ----------
            mm_sb = acts.tile([128, NT, COLS], F32, tag="mm")
            m2 = gatep.tile([128, MUL], F32, tag="m2")
            g = acts.tile([128, NT, COLS], F32, tag="g")
            fh = acts.tile([128, NT, COLS], F16, tag="fh")
            fl = acts.tile([128, NT, COLS], F16, tag="fl")
            for (t0, t1) in [(0, 2), (2, 4), (4, 6), (6, 8)]:
                for t in range(t0, t1):
                    drain_stats(t, mm_sb, m2, None, 0)
                gate_group(0, z0[:, t0:t1], t0, t1, m2, mm_sb, g, fh, fl,
                           True)

            # ---------- layers 1..5 ----------
            for b in range(5):
                l = b + 1
                wh, wl = load_w(b)
                mm_sb_n = acts.tile([128, NT, COLS], F32, tag="mm")
                m2_n = gatep.tile([128, MUL], F32, tag="m2")
                vps = [stats.tile([128, t1 - t0], F32, tag=f"vp{gi}",
                                  name=f"vp{l}_{gi}")
                       for gi, (t0, t1) in enumerate(GROUPS)]
                ag_outs = []
                for (u0, u1) in PHASES:
                    last_phase = (u1 == NT)
                    for t in range(NT):
                        for u in range(u0, u1):
                            lh = wh[:, u, t * 128:(t + 1) * 128]
                            ll = wl[:, u, t * 128:(t + 1) * 128]
                            nc.tensor.matmul(ps[:, t, 0:COLS], lh,
                                             fh[:, u, :],
                                             start=(u == 0), stop=False)
                            nc.tensor.matmul(ps[:, t, 0:COLS], lh,
                                             fl[:, u, :],
                                             start=False, stop=False)
                            nc.tensor.matmul(ps[:, t, 0:COLS], ll,
                                             fh[:, u, :],
                                             start=False, stop=(u == NT - 1))
                        if last_phase:
                            gi = next(i for i, (t0, t1) in enumerate(GROUPS)
                                      if t0 <= t < t1)
                            t0, t1 = GROUPS[gi]
                            drain_stats(t, mm_sb_n, m2_n, vps[gi], t - t0)
                            if t == t1 - 1:
                                ngr = t1 - t0
                                ag_i = dramp.tile([128, ngr], F32,
                                                  tag=f"agi{l}_{gi}")
                                ag_o = dramp.tile([NCORE, 128, ngr], F32,
                                                  tag=f"ago{l}_{gi}")
                                nc.scalar.dma_start(ag_i[:], vps[gi][:])
                                nc.gpsimd.collective_compute(
                                    "AllGather", ALU.bypass,
                                    replica_groups=[list(range(NCORE))],
                                    ins=[ag_i.opt()], outs=[ag_o.opt()])
                                ag_outs.append(ag_o)

                g_n = acts.tile([128, NT, COLS], F32, tag="g")
                fh_n = acts.tile([128, NT, COLS], F16, tag="fh")
                fl_n = acts.tile([128, NT, COLS], F16, tag="fl")
                for gi, (t0, t1) in enumerate(GROUPS):
                    ngr = t1 - t0
                    zcat = stats.tile([128, NCORE, ngr], F32, tag="zcat")
                    nc.scalar.dma_start(zcat[:],
                                        ag_outs[gi].rearrange("r p g -> p r g"))
                    z_g = stats.tile([128, ngr], F32, tag="zg")
                    nc.vector.tensor_reduce(
                        out=z_g[:], in_=zcat.rearrange("p r g -> p g r"),
                        axis=mybir.AxisListType.X, op=ALU.add)
                    gate_group(l, z_g[:], t0, t1, m2_n, mm_sb_n,
                               g_n, fh_n, fl_n, produce_split=(l < 5))
                mm_sb, m2, g, fh, fl = mm_sb_n, m2_n, g_n, fh_n, fl_n

            # ---------- output layer ----------
            for u in range(NT):
                nc.tensor.matmul(ps[0:1, 0, 0:COLS], wout_sb[:, u:u + 1],
                                 g[:, u, :], start=(u == 0), stop=(u == NT - 1))
            o_sb = stats.tile([1, COLS], F32, tag="o")
            nc.vector.tensor_copy(o_sb[:], ps[0:1, 0, 0:COLS])
            nc.sync.dma_start(d_out[:], o_sb[:])

    nc.compile()
    return nc


_NC = None


def _get_nc():
    global _NC
    if _NC is None:
        _NC = _build()
    return _NC


def _prep_in_maps(x, w1, W, bn_w, w_out):
    B = 1024
    xbui = x.reshape(B, 2, 3).astype(np.float32)
    xx = np.ascontiguousarray(xbui.transpose(1, 2, 0))     # [u, i, b]
    xq = np.ascontiguousarray(
        xbui.transpose(0, 2, 1).reshape(3072, 2).reshape(24, 128, 2))
    w1f = np.ascontiguousarray(w1.astype(np.float32))
    w1T = np.ascontiguousarray(w1f.T.reshape(NT, 128, 2))
    W0 = np.ascontiguousarray(W[:, 0].astype(np.float32))  # [5, u, v]
    Wh = W0.astype(np.float16)
    Wl = (W0 - Wh.astype(np.float32)).astype(np.float16)
    Wh = np.ascontiguousarray(Wh.reshape(5, NT, 128, MUL))
    Wl = np.ascontiguousarray(Wl.reshape(5, NT, 128, MUL))
    bnw = np.ascontiguousarray(
        bn_w[:, 0].astype(np.float32).reshape(NLAYER, NT, 128)
        .transpose(0, 2, 1))
    woutT = np.ascontiguousarray(
        (w_out[:, 0].astype(np.float32) / 32.0).reshape(NT, 128).T)

    in_maps = []
    for c in range(NCORE):
        x_c = np.ascontiguousarray(
            xx[:, :, c * BSH:(c + 1) * BSH].reshape(2, COLS))
        in_maps.append({"x_c": x_c, "xq": xq, "w1": w1f, "w1T": w1T,
                        "wh": Wh, "wl": Wl, "bnw": bnw, "woutT": woutT})
    return in_maps


def _run(inputs, trace=False, trace_cores=None):
    nc = _get_nc()
    in_maps = _prep_in_maps(inputs["x"], inputs["w1"], inputs["W"],
                            inputs["bn_w"], inputs["w_out"])
    res = run_bass_kernel_spmd(nc, in_maps, core_ids=list(range(NCORE)),
                               trace=trace, trace_cores=trace_cores)
    out = np.empty((1024, 3), dtype=np.float32)
    for c in range(NCORE):
        o = res.results[c]["out"].reshape(3, 128).T
        out[c * BSH:(c + 1) * BSH] = o
    return out, res


def kernel(**inputs) -> np.ndarray:
    out, _ = _run(inputs, trace=False)
    return out


# revision 17
# speedup vs baseline: 1.2560x; 1.0403x over previous
"""Trainium2 Bass kernel for nn_E3nnMLPNorm (8-core SPMD).

Structure exploited: the input irreps are '2x1e' and every linear is
block-diagonal per irrep, so the l=2,3,4 fields are exactly zero through
the whole network (bn_act(0) == 0). Only the l=1 (d=3) path is computed.

Sharding: data-parallel over batch B=1024 -> 128 rows/core. Activations
live as [v(128-partition tiles), col = i*128 + b_local].

Per layer (1..5): mm = f @ W on PE as fp16 hi/lo 3-pass (fp32 PSUM
accumulate), ordered in u-phases {0,1},{2,3},{4..7} so the next layer can
start as soon as the first normalized u-tiles arrive. Banks complete
t-major inside the last phase; each finished bank is immediately drained
to SBUF (freeing PSUM for the next layer) while ACT computes sum(mm^2).
Batch-norm statistics cross cores via 3 pipelined AllGathers (4/2/2KB) +
local reduce — cheaper floor than AllReduce and overlapped with the PE.
Layer 0 needs no collective at all: var0 is a quadratic form in the
2x2 Gram matrix of the full x, computed locally on every core.
"""
import sys, types
sys.path.insert(0, "/opt/trn_rl_repo")
import numpy as np

# ---- shim antenv.axon_hooks so trace=True works under axon ----
if "antenv.axon_hooks" not in sys.modules:
    _hook_store = {}
    _m = types.ModuleType("antenv.axon_hooks")
    _m.set_axon_ntff_profile_hook = lambda h: _hook_store.__setitem__("h", h)
    _m.get_axon_ntff_profile_hook = lambda: _hook_store.get("h")
    sys.modules["antenv.axon_hooks"] = _m
    try:
        import antenv
        antenv.axon_hooks = _m
        from trn_agent_boot.trn_boot import _ntff_profile_via_ctypes
        _m.set_axon_ntff_profile_hook(
            _ntff_profile_via_ctypes("/opt/axon/libaxon_pjrt.so"))
    except Exception:
        pass

import concourse.bass as bass
import concourse.bacc as bacc
import concourse.mybir as mybir
import concourse.tile as tile
import concourse.bass_utils as bass_utils
bass_utils.upload_artifacts = lambda tmpdir: tmpdir
from concourse.bass_utils import run_bass_kernel_spmd

F32 = mybir.dt.float32
F16 = mybir.dt.float16
ALU = mybir.AluOpType
ACT = mybir.ActivationFunctionType

NCORE = 8
MUL = 1024
NT = 8
BSH = 128
COLS = 3 * BSH
NLAYER = 6

# stat groups (tile ranges) and matching u-phases
GROUPS = [(0, 2), (2, 4), (4, 8)]
PHASES = [(0, 2), (2, 4), (4, 8)]

EPS_L = [2e-5] + [1.024e-2] * 5      # rsqrt eps with 1/sqrt(mul) folded in
INV_SAMPLES = 1.0 / (MUL * 3)


def _build():
    nc = bacc.Bacc("TRN2", target_bir_lowering=False, debug=False,
                   enable_asserts=True, num_devices=NCORE)

    d_x = nc.dram_tensor("x_c", [2, COLS], F32, kind="ExternalInput")
    d_xq = nc.dram_tensor("xq", [24, 128, 2], F32, kind="ExternalInput")
    d_w1 = nc.dram_tensor("w1", [2, MUL], F32, kind="ExternalInput")
    d_w1T = nc.dram_tensor("w1T", [NT, 128, 2], F32, kind="ExternalInput")
    d_wh = nc.dram_tensor("wh", [5, NT, 128, MUL], F16, kind="ExternalInput")
    d_wl = nc.dram_tensor("wl", [5, NT, 128, MUL], F16, kind="ExternalInput")
    d_bnw = nc.dram_tensor("bnw", [NLAYER, 128, NT], F32, kind="ExternalInput")
    d_wout = nc.dram_tensor("woutT", [128, NT], F32, kind="ExternalInput")
    d_out = nc.dram_tensor("out", [1, COLS], F32, kind="ExternalOutput")

    with tile.TileContext(nc) as tc:
        with tc.tile_pool(name="const", bufs=1) as constp, \
             tc.tile_pool(name="wpool", bufs=3) as wpool, \
             tc.tile_pool(name="acts", bufs=2) as acts, \
             tc.tile_pool(name="acts1", bufs=1) as acts1, \
             tc.tile_pool(name="sqp", bufs=3) as sqp, \
             tc.tile_pool(name="stats", bufs=3) as stats, \
             tc.tile_pool(name="gate", bufs=2) as gatep, \
             tc.tile_pool(name="psum", bufs=1, space="PSUM") as psump, \
             tc.tile_pool(name="dram", bufs=1, space="DRAM") as dramp:

            # align cores up-front so the first real collective doesn't
            # absorb the cross-core launch skew
            sync_in = dramp.tile([128, 1], F32, tag="sync_i")
            sync_out = dramp.tile([NCORE, 128, 1], F32, tag="sync_o")
            nc.gpsimd.collective_compute(
                "AllGather", ALU.bypass,
                replica_groups=[list(range(NCORE))],
                ins=[sync_in.opt()], outs=[sync_out.opt()])

            # ---- static loads ----
            x_sb = constp.tile([2, COLS], F32, tag="x")
            nc.sync.dma_start(x_sb[:], d_x[:])
            xq_sb = constp.tile([128, 24, 2], F32, tag="xq")
            nc.sync.dma_start(xq_sb[:], d_xq.rearrange("c p j -> p c j"))
            w1_sb = constp.tile([2, MUL], F32, tag="w1")
            nc.sync.dma_start(w1_sb[:], d_w1[:])
            w1T_sb = constp.tile([128, NT, 2], F32, tag="w1T")
            nc.sync.dma_start(w1T_sb[:], d_w1T.rearrange("t p j -> p t j"))
            bnw_sb = constp.tile([128, NLAYER, NT], F32, tag="bnw")
            nc.sync.dma_start(bnw_sb[:], d_bnw.rearrange("l p t -> p l t"))
            wout_sb = constp.tile([128, NT], F32, tag="wout")
            nc.sync.dma_start(wout_sb[:], d_wout[:])
            eps8 = constp.tile([128, 1], F32, tag="eps8")
            nc.vector.memset(eps8[:], 1e-8)

            ps = psump.tile([128, NT, 512], F32, tag="ps")

            def load_w(b):
                wh = wpool.tile([128, NT, MUL], F16, tag="wh")
                nc.sync.dma_start(wh[:], d_wh[b].rearrange("t p v -> p t v"))
                wl = wpool.tile([128, NT, MUL], F16, tag="wl")
                nc.sync.dma_start(wl[:], d_wl[b].rearrange("t p v -> p t v"))
                return wh, wl

            # ---------- layer 0: Gram-trick stats (no collective) ----------
            # S = sum_c x x^T over all 3072 samples, via 24 K=128 matmuls
            for c in range(24):
                nc.tensor.matmul(ps[0:2, 0, 0:2], xq_sb[:, c, :],
                                 xq_sb[:, c, :], start=(c == 0),
                                 stop=(c == 23))
            S_sb = stats.tile([2, 2], F32, tag="S")
            nc.vector.tensor_copy(S_sb[:], ps[0:2, 0, 0:2])
            sv = stats.tile([1, 3], F32, tag="sv")
            nc.sync.dma_start(sv[0:1, 0:2], S_sb[0:1, 0:2])
            nc.sync.dma_start(sv[0:1, 2:3], S_sb[1:2, 1:2])
            S_brd = stats.tile([128, 3], F32, tag="Sbrd")
            nc.gpsimd.partition_broadcast(S_brd[:], sv[0:1, :])
            A = w1T_sb[:, :, 0]
            B = w1T_sb[:, :, 1]
            AA = stats.tile([128, NT], F32, tag="AA")
            nc.vector.tensor_mul(AA[:], A, A)
            AB2 = stats.tile([128, NT], F32, tag="AB2")
            nc.vector.scalar_tensor_tensor(out=AB2[:], in0=A, scalar=2.0,
                                           in1=B, op0=ALU.mult, op1=ALU.mult)
            BB = stats.tile([128, NT], F32, tag="BB")
            nc.vector.tensor_mul(BB[:], B, B)
            z0 = stats.tile([128, NT], F32, tag="z0")
            nc.vector.tensor_scalar_mul(out=z0[:], in0=AA[:],
                                        scalar1=S_brd[:, 0:1])
            nc.vector.scalar_tensor_tensor(out=z0[:], in0=AB2[:],
                                           scalar=S_brd[:, 1:2], in1=z0[:],
                                           op0=ALU.mult, op1=ALU.add)
            nc.vector.scalar_tensor_tensor(out=z0[:], in0=BB[:],
                                           scalar=S_brd[:, 2:3], in1=z0[:],
                                           op0=ALU.mult, op1=ALU.add)

            # layer-0 matmul: [2]x[2,128v] fp32
            for t in range(NT):
                nc.tensor.matmul(ps[:, t, 0:COLS],
                                 w1_sb[:, t * 128:(t + 1) * 128],
                                 x_sb[:], start=True, stop=True)

            # ---------- shared per-layer pieces ----------
            def drain_stats(t, mm_sb, m2, vp, vcol):
                mm_t = ps[:, t, 0:COLS]
                sq = sqp.tile([128, COLS], F32, tag="sq")
                if vp is not None:
                    nc.scalar.activation(sq[:], mm_t, ACT.Square,
                                         accum_out=vp[:, vcol:vcol + 1])
                else:
                    nc.scalar.activation(sq[:], mm_t, ACT.Square)
                nc.scalar.copy(mm_sb[:, t, :], mm_t)
                nc.vector.tensor_reduce(
                    out=m2[:, t * 128:(t + 1) * 128],
                    in_=sq.rearrange("p (i b) -> p b i", i=3),
                    axis=mybir.AxisListType.X, op=ALU.add)

            def gate_group(l, z_g, t0, t1, m2, mm_sb, g, fh, fl,
                           produce_split):
                ng = t1 - t0
                zz = stats.tile([128, ng], F32, tag="zz")
                nc.vector.tensor_scalar(out=zz[:], in0=z_g,
                                        scalar1=INV_SAMPLES, scalar2=EPS_L[l],
                                        op0=ALU.mult, op1=ALU.add)
                rr = stats.tile([128, ng], F32, tag="rr")
                nc.vector.reciprocal(rr[:], zz[:])
                spre = stats.tile([128, ng], F32, tag="spre")
                nc.scalar.sqrt(spre[:], rr[:])
                s_g = stats.tile([128, ng], F32, tag="sg")
                nc.vector.tensor_mul(s_g[:], spre[:], bnw_sb[:, l, t0:t1])
                s2_g = stats.tile([128, ng], F32, tag="s2g")
                nc.vector.tensor_mul(s2_g[:], s_g[:], s_g[:])

                gc = slice(t0 * 128, t1 * 128)
                t1v = gatep.tile([128, ng * 128], F32, tag="t1v")
                for i, t in enumerate(range(t0, t1)):
                    nc.vector.tensor_scalar_mul(
                        out=t1v[:, i * 128:(i + 1) * 128],
                        in0=m2[:, t * 128:(t + 1) * 128],
                        scalar1=s2_g[:, i:i + 1])
                n_g = gatep.tile([128, ng * 128], F32, tag="ng")
                nc.scalar.activation(n_g[:], t1v[:], ACT.Sqrt,
                                     bias=eps8[:], scale=1.0)
                sig = gatep.tile([128, ng * 128], F32, tag="sig")
                nc.scalar.activation(sig[:], n_g[:], ACT.Sigmoid)
                for i, t in enumerate(range(t0, t1)):
                    rn = gatep.tile([128, 128], F32, tag="rn")
                    nc.vector.reciprocal(rn[:], n_g[:, i * 128:(i + 1) * 128])
                    Qc = gatep.tile([128, 128], F32, tag="Qc")
                    nc.vector.scalar_tensor_tensor(
                        out=Qc[:], in0=sig[:, i * 128:(i + 1) * 128],
                        scalar=s_g[:, i:i + 1], in1=rn[:],
                        op0=ALU.mult, op1=ALU.mult)
                    mm_v = mm_sb[:, t, :].rearrange("p (i b) -> p i b", i=3)
                    g_v = g[:, t, :].rearrange("p (i b) -> p i b", i=3)
                    qb = Qc[:, None, :].broadcast_to([128, 3, 128])
                    nc.vector.tensor_tensor(out=g_v, in0=mm_v, in1=qb,
                                            op=ALU.mult)
                    if produce_split:
                        nc.scalar.copy(out=fh[:, t, :], in_=g[:, t, :])
                        nc.vector.tensor_sub(out=fl[:, t, :], in0=g[:, t, :],
                                             in1=fh[:, t, :])

            # ---------- layer 0 normalize (local stats, 2 groups) ----------
            mm_sb = acts1.tile([128, NT, COLS], F32, tag="mm")
            m2 = acts1.tile([128, MUL], F32, tag="m2")
            g = acts1.tile([128, NT, COLS], F32, tag="g")
            fh = acts.tile([128, NT, COLS], F16, tag="fh")
            fl = acts.tile([128, NT, COLS], F16, tag="fl")
            for (t0, t1) in [(0, 2), (2, 8)]:
                for t in range(t0, t1):
                    drain_stats(t, mm_sb, m2, None, 0)
                gate_group(0, z0[:, t0:t1], t0, t1, m2, mm_sb, g, fh, fl,
                           True)

            # ---------- layers 1..5 ----------
            for b in range(5):
                l = b + 1
                wh, wl = load_w(b)
                mm_sb_n = acts1.tile([128, NT, COLS], F32, tag="mm")
                m2_n = acts1.tile([128, MUL], F32, tag="m2")
                vps = [stats.tile([128, t1 - t0], F32, tag=f"vp{gi}",
                                  name=f"vp{l}_{gi}")
                       for gi, (t0, t1) in enumerate(GROUPS)]
                ag_outs = []
                for (u0, u1) in PHASES:
                    last_phase = (u1 == NT)
                    for t in range(NT):
                        for u in range(u0, u1):
                            lh = wh[:, u, t * 128:(t + 1) * 128]
                            ll = wl[:, u, t * 128:(t + 1) * 128]
                            nc.tensor.matmul(ps[:, t, 0:COLS], lh,
                                             fh[:, u, :],
                                             start=(u == 0), stop=False)
                            nc.tensor.matmul(ps[:, t, 0:COLS], lh,
                                             fl[:, u, :],
                                             start=False, stop=False)
                            nc.tensor.matmul(ps[:, t, 0:COLS], ll,
                                             fh[:, u, :],
                                             start=False, stop=(u == NT - 1))
                        if last_phase:
                            gi = next(i for i, (t0, t1) in enumerate(GROUPS)
                                      if t0 <= t < t1)
                            t0, t1 = GROUPS[gi]
                            drain_stats(t, mm_sb_n, m2_n, vps[gi], t - t0)
                            if t == t1 - 1:
                                ngr = t1 - t0
                                ag_i = dramp.tile([128, ngr], F32,
                                                  tag=f"agi{l}_{gi}")
                                ag_o = dramp.tile([NCORE, 128, ngr], F32,
                                                  tag=f"ago{l}_{gi}")
                                nc.scalar.dma_start(ag_i[:], vps[gi][:])
                                nc.gpsimd.collective_compute(
                                    "AllGather", ALU.bypass,
                                    replica_groups=[list(range(NCORE))],
                                    ins=[ag_i.opt()], outs=[ag_o.opt()])
                                ag_outs.append(ag_o)

                g_n = acts1.tile([128, NT, COLS], F32, tag="g")
                fh_n = acts.tile([128, NT, COLS], F16, tag="fh")
                fl_n = acts.tile([128, NT, COLS], F16, tag="fl")
                for gi, (t0, t1) in enumerate(GROUPS):
                    ngr = t1 - t0
                    zcat = stats.tile([128, NCORE, ngr], F32, tag="zcat")
                    nc.scalar.dma_start(zcat[:],
                                        ag_outs[gi].rearrange("r p g -> p r g"))
                    z_g = stats.tile([128, ngr], F32, tag="zg")
                    nc.vector.tensor_reduce(
                        out=z_g[:], in_=zcat.rearrange("p r g -> p g r"),
                        axis=mybir.AxisListType.X, op=ALU.add)
                    gate_group(l, z_g[:], t0, t1, m2_n, mm_sb_n,
                               g_n, fh_n, fl_n, produce_split=(l < 5))
                mm_sb, m2, g, fh, fl = mm_sb_n, m2_n, g_n, fh_n, fl_n

            # ---------- output layer ----------
            for u in range(NT):
                nc.tensor.matmul(ps[0:1, 0, 0:COLS], wout_sb[:, u:u + 1],
                                 g[:, u, :], start=(u == 0), stop=(u == NT - 1))
            o_sb = stats.tile([1, COLS], F32, tag="o")
            nc.vector.tensor_copy(o_sb[:], ps[0:1, 0, 0:COLS])
            nc.sync.dma_start(d_out[:], o_sb[:])

    nc.compile()
    return nc


_NC = None


def _get_nc():
    global _NC
    if _NC is None:
        _NC = _build()
    return _NC


def _prep_in_maps(x, w1, W, bn_w, w_out):
    B = 1024
    xbui = x.reshape(B, 2, 3).astype(np.float32)
    xx = np.ascontiguousarray(xbui.transpose(1, 2, 0))     # [u, i, b]
    xq = np.ascontiguousarray(
        xbui.transpose(0, 2, 1).reshape(3072, 2).reshape(24, 128, 2))
    w1f = np.ascontiguousarray(w1.astype(np.float32))
    w1T = np.ascontiguousarray(w1f.T.reshape(NT, 128, 2))
    W0 = np.ascontiguousarray(W[:, 0].astype(np.float32))  # [5, u, v]
    Wh = W0.astype(np.float16)
    Wl = (W0 - Wh.astype(np.float32)).astype(np.float16)
    Wh = np.ascontiguousarray(Wh.reshape(5, NT, 128, MUL))
    Wl = np.ascontiguousarray(Wl.reshape(5, NT, 128, MUL))
    bnw = np.ascontiguousarray(
        bn_w[:, 0].astype(np.float32).reshape(NLAYER, NT, 128)
        .transpose(0, 2, 1))
    woutT = np.ascontiguousarray(
        (w_out[:, 0].astype(np.float32) / 32.0).reshape(NT, 128).T)

    in_maps = []
    for c in range(NCORE):
        x_c = np.ascontiguousarray(
            xx[:, :, c * BSH:(c + 1) * BSH].reshape(2, COLS))
        in_maps.append({"x_c": x_c, "xq": xq, "w1": w1f, "w1T": w1T,
                        "wh": Wh, "wl": Wl, "bnw": bnw, "woutT": woutT})
    return in_maps


def _run(inputs, trace=False, trace_cores=None):
    nc = _get_nc()
    in_maps = _prep_in_maps(inputs["x"], inputs["w1"], inputs["W"],
                            inputs["bn_w"], inputs["w_out"])
    res = run_bass_kernel_spmd(nc, in_maps, core_ids=list(range(NCORE)),
                               trace=trace, trace_cores=trace_cores)
    out = np.empty((1024, 3), dtype=np.float32)
    for c in range(NCORE):
        o = res.results[c]["out"].reshape(3, 128).T
        out[c * BSH:(c + 1) * BSH] = o
    return out, res


def kernel(**inputs) -> np.ndarray:
    out, _ = _run(inputs, trace=False)
    return out
